# revision 21
# baseline (speedup 1.0000x reference)
"""Distributed MultiHeadAttention kernel for 8 Trainium2 NeuronCores.

Problem: B=2, L=2048, D=1024, H=16 heads (DH=64), causal attn_mask +
key_padding_mask, torch-Linear-convention projections.

Sharding: core = (batch b = core//4, group rank j = core%4). Each core
projects q/k/v for its batch restricted to its 4 heads (256 channels),
runs streaming softmax attention in a [key, query]-transposed layout
(no max subtraction -- scores are O(1); masked scores underflow exp to
exactly 0), AllGathers the raw attention + row sums within each 4-core
group in four (head-pair, query-chunk) pieces, and computes the output
projection for its own 512 rows. Host assembles [2, 2048, 1024].

Structure is built around the ACT engine's exp chain (the critical
resource: exp throughput is a fixed 1 elem/lane/cycle): scores for the
first query chunk start as soon as two projection chunks have landed,
projection matmuls for the remaining chunks fill the PE between
ACT-paced attention matmuls, and the key-padding mask is folded into
the exp's per-partition bias operand (keys are partitions in the
transposed layout) so only causal diagonal tiles need vector adds.
Row sums ride in the attention AllGather payload (fp32 rows bitcast
into the bf16 buffer). Matmuls run in bf16; accumulation fp32.
"""
import os
import sys

sys.path.insert(0, '/opt/trn_rl_repo')

import numpy as np
import ml_dtypes

import concourse.bass as bass
import concourse.bacc as bacc
import concourse.mybir as mybir
import concourse.tile as tile
from concourse.bass_utils import run_bass_kernel_spmd

BF16 = mybir.dt.bfloat16
F32 = mybir.dt.float32
NPBF16 = ml_dtypes.bfloat16

B, L, D, H = 2, 2048, 1024, 16
DH = D // H                      # 64
N_CORES = 8
GROUPS = [[0, 1, 2, 3], [4, 5, 6, 7]]
HPC = H // 4                     # heads per core = 4
CPC = HPC * DH                   # channels per core = 256
LPC = L // 4                     # output rows per core = 512
QC = 1024                        # query-chunk size
NQC = L // QC                    # 2
KB = 128                         # key-block size
NKB = L // KB                    # 16
NLC = 4                          # projection l-chunks of 512
NDB = D // 128                   # 8 contraction blocks
MASK_VAL = -1e5                  # pre-scale additive causal mask
KPM_BIAS = -1e4                  # post-scale exp bias for padded keys
AGW = 132                        # AG rows: 128 attn + 4 S rows (fp32 pairs)

ExpFn = mybir.ActivationFunctionType.Exp

_PROG_CACHE = {}
last_results = None


def _analyze_masks(attn_mask, key_padding_mask):
    """Derive the (qc, kb) tile structure, shared causal 2D mask tiles
    (from attn_mask only) and per-batch per-key exp-bias columns (from
    key_padding_mask)."""
    am = np.asarray(attn_mask, dtype=bool)
    kpm = np.asarray(key_padding_mask, dtype=bool)
    cm = [am | kpm[b][None, :] for b in range(B)]     # [L, L], True = masked

    for b in range(B):
        if cm[b].all(axis=1).any():
            return None, None, None, True

    structure = []
    mask_chunks = []
    off = 0
    for qc in range(NQC):
        recs = []
        for kb in range(NKB):
            subs = [cm[b][qc * QC:(qc + 1) * QC, kb * KB:(kb + 1) * KB]
                    for b in range(B)]                 # [QC, 128]
            allowed = [~s.all(axis=1) for s in subs]
            union = allowed[0] | allowed[1]
            if not union.any():
                continue
            q0 = int(np.argmax(union))
            if not union[q0:].all():
                q0 = 0
            am_sub = am[qc * QC + q0:(qc + 1) * QC, kb * KB:(kb + 1) * KB]
            acols = am_sub.any(axis=1)
            mask_rec = None
            if acols.any():
                c0 = q0 + int(np.argmax(acols))
                c1 = q0 + len(acols) - int(np.argmax(acols[::-1]))
                w = c1 - c0
                tileM = np.where(
                    am[qc * QC + c0:qc * QC + c1,
                       kb * KB:(kb + 1) * KB].T,
                    np.float32(MASK_VAL), np.float32(0.0))  # [128, w]
                mask_chunks.append(tileM)
                mask_rec = (off, c0, w)
                off += w
            recs.append((kb, q0, mask_rec))
        if not recs:
            return None, None, None, True
        started = [False, False]
        for kb, q0, _ in recs:
            for s in range(QC // 512):
                lo, hi = max(q0, s * 512), (s + 1) * 512
                if lo < hi and not started[s]:
                    if lo != s * 512:
                        return None, None, None, True
                    started[s] = True
        structure.append(recs)

    mw = max(off, 1)
    mask_buf = np.zeros((128, mw), dtype=np.float32)
    o = 0
    for tileM in mask_chunks:
        mask_buf[:, o:o + tileM.shape[1]] = tileM
        o += tileM.shape[1]

    kbias = []
    for b in range(B):
        kb_b = np.zeros((128, NKB), dtype=np.float32)
        for kb in range(NKB):
            kb_b[:, kb] = np.where(kpm[b, kb * KB:(kb + 1) * KB],
                                   np.float32(KPM_BIAS), np.float32(0.0))
        kbias.append(kb_b)
    return structure, mask_buf, kbias, False


def _structure_key(structure, mw):
    return (mw, tuple(tuple((kb, q0, mask) for kb, q0, mask in recs)
                      for recs in structure))


def _build_program(structure, mw):
    """Build the SPMD Bass program (identical on all 8 cores)."""
    nc = bacc.Bacc("TRN2", target_bir_lowering=False, debug=False,
                   num_devices=N_CORES)

    xq_c = nc.declare_dram_parameter("xq_c", [NLC, 128, NDB * 512], BF16,
                                     isOutput=False)
    xk_c = nc.declare_dram_parameter("xk_c", [NLC, 128, NDB * 512], BF16,
                                     isOutput=False)
    xv_c = nc.declare_dram_parameter("xv_c", [NLC, 128, NDB * 512], BF16,
                                     isOutput=False)
    wqT = nc.declare_dram_parameter("wqT", [D, CPC], BF16, isOutput=False)
    wkT = nc.declare_dram_parameter("wkT", [D, CPC], BF16, isOutput=False)
    wvT = nc.declare_dram_parameter("wvT", [D, CPC], BF16, isOutput=False)
    woT = nc.declare_dram_parameter("woT", [D, D], BF16, isOutput=False)
    bq_in = nc.declare_dram_parameter("bq", [128, 2], F32, isOutput=False)
    bk_in = nc.declare_dram_parameter("bk", [128, 2], F32, isOutput=False)
    bv_in = nc.declare_dram_parameter("bv", [1, CPC], BF16, isOutput=False)
    bo_in = nc.declare_dram_parameter("bo", [1, D], BF16, isOutput=False)
    masks_in = nc.declare_dram_parameter("masks", [128, mw], F32,
                                         isOutput=False)
    kbias_in = nc.declare_dram_parameter("kbias", [128, NKB], F32,
                                         isOutput=False)
    out = nc.declare_dram_parameter("out", [LPC, D], F32, isOutput=True)

    # AllGather pieces: one buffer per (head-pair p, query-chunk qc).
    # Logical layout [AGW, QC] bf16: rows 0-127 raw attention; rows
    # 128-131 (fp32 rows 128+hp*2+s in the bitcast view) row sums S.
    # Declared with 4096-wide rows so the collective moves 8KB lines.
    ag_in = {(p, qc): nc.dram_tensor(f"ag_in{p}_{qc}", [AGW * QC // 4096, 4096],
                                     BF16)
             for p in range(2) for qc in range(NQC)}
    ag_out = [nc.dram_tensor(f"ag_out{p}", [NQC, 4, AGW * QC // 4096, 4096],
                             BF16)
              for p in range(2)]
    ag_out_f32 = [t.bitcast(F32) for t in ag_out]
    r_dram = [nc.dram_tensor(f"r_dram{p}", [8, 512], F32) for p in range(2)]
    wu_in = nc.dram_tensor("wu_in", [8, 64], BF16)
    wu_out = nc.dram_tensor("wu_out", [4, 8, 64], BF16)

    with tile.TileContext(nc, num_cores=N_CORES) as tc:
        with tc.tile_pool(name="persist", bufs=1) as pers:
            wq_sb = pers.tile([128, NDB, CPC], BF16, tag="wq")
            wk_sb = pers.tile([128, NDB, CPC], BF16, tag="wk")
            wv_sb = pers.tile([128, NDB, CPC], BF16, tag="wv")
            wo_sb = pers.tile([128, NDB, D], BF16, tag="wo")
            bq_sb = pers.tile([128, 2], F32, tag="bq")
            bk_sb = pers.tile([128, 2], F32, tag="bk")
            bv_sb = pers.tile([1, CPC], BF16, tag="bv")
            bo_sb = pers.tile([1, D], BF16, tag="bo")
            masks_sb = pers.tile([128, mw], F32, tag="masks")
            kbias_sb = pers.tile([128, NKB], F32, tag="kbias")
            ones_sb = pers.tile([1, 128], BF16, tag="ones")
            qT_sb = pers.tile([128, 2, L], BF16, tag="qT")
            kT_sb = pers.tile([128, 2, L], BF16, tag="kT")
            v_sb = pers.tile([128, NKB, HPC, DH + 1], BF16, tag="v")
            heat_sb = pers.tile([128, 1024], BF16, tag="heat")

            # priority-ordered input pushes (sync queue = input stream)
            nc.sync.dma_start(
                out=wk_sb[:], in_=wkT.ap().rearrange("(db p) c -> p db c", p=128))
            nc.sync.dma_start(
                out=wq_sb[:], in_=wqT.ap().rearrange("(db p) c -> p db c", p=128))
            nc.sync.dma_start(out=bq_sb[:], in_=bq_in[:])
            nc.sync.dma_start(out=bk_sb[:], in_=bk_in[:])
            nc.vector.memset(ones_sb[:], 1.0)
            nc.vector.memset(v_sb[:, :, :, DH:DH + 1], 1.0)
            nc.vector.memset(heat_sb[:], 0.001)

            pid_s = nc.sync.partition_id()
            j_s = pid_s % 4
            pid_c = nc.scalar.partition_id()
            j_c = pid_c % 4

            # PE heater: bridges the input-DMA ramp so HAM stays warm
            with tc.tile_pool(name="psH", bufs=1, space="PSUM") as psH:
                hps = psH.tile([128, 512], F32, tag="hps")
                for it in range(34):
                    nc.tensor.matmul(hps[:], lhsT=heat_sb[:, 0:128],
                                     rhs=heat_sb[:, 512:1024],
                                     start=(it == 0), stop=False)
                for it in range(24):
                    nc.tensor.matmul(hps[:, 0:128], lhsT=heat_sb[:, 0:128],
                                     rhs=heat_sb[:, 512:640],
                                     start=False, stop=(it == 23))

            # CC-stream warmup collective
            nc.gpsimd.dma_start(out=wu_in[:], in_=heat_sb[0:8, 0:64])
            nc.gpsimd.collective_compute(
                "AllGather", mybir.AluOpType.bypass, replica_groups=GROUPS,
                ins=[wu_in[:]], outs=[wu_out[:]])

            with tc.tile_pool(name="xt", bufs=2) as xtp, \
                 tc.tile_pool(name="ex", bufs=4) as ex_pool, \
                 tc.tile_pool(name="ship", bufs=2) as ship_pool, \
                 tc.tile_pool(name="oph", bufs=1) as oph:

                # PSUM pools with explicit lifetimes (8-bank budget):
                # ramp+qc0: psS(4) + psP(2) + psA0(2); qc1: psS(4) + psA1(4);
                # phase O: psO(8).
                psS = tc.alloc_tile_pool(name="psS", bufs=2, space="PSUM")
                psP = tc.alloc_tile_pool(name="psP", bufs=2, space="PSUM")
                psA0 = tc.alloc_tile_pool(name="psA0", bufs=1, space="PSUM")

                xts = {}

                def emit_load(lc):
                    for nm, dram in (("xtk", xk_c), ("xtq", xq_c),
                                     ("xtv", xv_c)):
                        t = xtp.tile([128, NDB, 512], BF16, tag=nm,
                                     name=f"{nm}_{lc}")
                        nc.sync.dma_start(
                            out=t[:],
                            in_=dram[lc].rearrange("p (db l) -> p db l",
                                                   db=NDB))
                        xts[(nm, lc)] = t

                def emit_proj_qk(lc, w_sb, b_sb, t_sb, xnm):
                    x_sb = xts[(xnm, lc)]
                    for cb in range(2):
                        ps = psP.tile([128, 512], F32, tag="pp",
                                      name=f"pp_{xnm}_{lc}_{cb}")
                        for db in range(NDB):
                            nc.tensor.matmul(
                                ps[:],
                                lhsT=w_sb[:, db, cb * 128:(cb + 1) * 128],
                                rhs=x_sb[:, db, :],
                                start=(db == 0), stop=(db == NDB - 1))
                        nc.vector.tensor_scalar_add(
                            t_sb[:, cb, lc * 512:(lc + 1) * 512], ps[:],
                            b_sb[:, cb:cb + 1])

                def emit_proj_v(lc):
                    x_sb = xts[("xtv", lc)]
                    for ls in range(4):
                        kbg = lc * 4 + ls
                        ps = psP.tile([128, 512], F32, tag="pp",
                                      name=f"pv_{lc}_{ls}")
                        psv = ps[:, 0:CPC]
                        for db in range(NDB):
                            nc.tensor.matmul(
                                psv,
                                lhsT=x_sb[:, db, ls * 128:(ls + 1) * 128],
                                rhs=wv_sb[:, db, :],
                                start=(db == 0), stop=False)
                        nc.tensor.matmul(
                            psv, lhsT=ones_sb[:, 0:128], rhs=bv_sb[:],
                            start=False, stop=True)
                        nc.vector.tensor_copy(
                            v_sb[:, kbg, :, 0:DH],
                            psv.rearrange("p (h d) -> p h d", h=HPC))

                def segs_of(q0):
                    out_ = []
                    for s in range(QC // 512):
                        lo, hi = max(q0, s * 512), (s + 1) * 512
                        if lo < hi:
                            out_.append((s, lo, hi))
                    return out_

                def emit_attnv(pa, kb, q0, ex, seg_first, seg_last, h):
                    for s, lo, hi in segs_of(q0):
                        nc.tensor.matmul(
                            pa[:, lo:hi], lhsT=v_sb[:, kb, h, :],
                            rhs=ex[:, lo:hi],
                            start=(seg_first[s] == kb),
                            stop=(seg_last[s] == kb))

                def emit_substream(qc, p, hp, psA_pool, araw):
                    recs = structure[qc]
                    h = p * 2 + hp
                    hb, hoff = p, hp * 64
                    seg_first, seg_last = {}, {}
                    for kb, q0, _m in recs:
                        for s, lo, hi in segs_of(q0):
                            seg_first.setdefault(s, kb)
                            seg_last[s] = kb
                    pa = psA_pool.tile([65, QC], F32, tag="pa",
                                       name=f"pa_{qc}_{p}_{hp}")
                    pend = None
                    for kb, q0, mask in recs:
                        ps = psS.tile([128, QC], F32, tag="ps",
                                      name=f"ps_{qc}_{p}_{hp}_{kb}")
                        for s, lo, hi in segs_of(q0):
                            nc.tensor.matmul(
                                ps[:, lo:hi],
                                lhsT=kT_sb[hoff:hoff + 64, hb,
                                           kb * KB:(kb + 1) * KB],
                                rhs=qT_sb[hoff:hoff + 64, hb,
                                          qc * QC + lo:qc * QC + hi],
                                start=True, stop=True)
                        if mask is not None:
                            off, c0, wm = mask
                            nc.vector.tensor_add(
                                ps[:, c0:c0 + wm], ps[:, c0:c0 + wm],
                                masks_sb[:, off:off + wm])
                        ex = ex_pool.tile([128, QC], BF16, tag="ex",
                                          name=f"ex_{qc}_{p}_{hp}_{kb}")
                        nc.scalar.activation(
                            out=ex[:, q0:], in_=ps[:, q0:], func=ExpFn,
                            scale=0.125, bias=kbias_sb[:, kb:kb + 1])
                        if pend is not None:
                            emit_attnv(*pend)
                        pend = (pa, kb, q0, ex, seg_first, seg_last, h)
                    emit_attnv(*pend)
                    # evacuate: attn rows -> araw (bf16), S row -> stmp
                    nc.vector.tensor_copy(araw[:, hp, :], pa[0:64, :])
                    stmp = ship_pool.tile([65, QC], F32, tag="stmp",
                                          name=f"stmp_{qc}_{p}_{hp}")
                    nc.vector.tensor_copy(stmp[64:65, :], pa[64:65, :])
                    agf = ag_in[(p, qc)].bitcast(F32)
                    for s in range(2):
                        nc.gpsimd.dma_start(
                            out=bass.AP(tensor=agf,
                                        offset=128 * 512 +
                                        (hp * 2 + s) * 512,
                                        ap=[[512, 1], [1, 512]]),
                            in_=stmp[64:65, s * 512:(s + 1) * 512])

                def emit_substream_paired(qc, p, psA_pool, araw):
                    """Both heads of pair p interleaved per key block:
                    the two score matmuls occupy disjoint 64-row halves
                    of the PE array (auto row tiling) and co-execute."""
                    recs = structure[qc]
                    hb = p
                    seg_first, seg_last = {}, {}
                    for kb, q0, _m in recs:
                        for s, lo, hi in segs_of(q0):
                            seg_first.setdefault(s, kb)
                            seg_last[s] = kb
                    pa = {hp: psA_pool.tile([65, QC], F32, tag="pa",
                                            name=f"pa_{qc}_{p}_{hp}")
                          for hp in range(2)}
                    pend = None
                    for kb, q0, mask in recs:
                        ps = {hp: psS.tile([128, QC], F32, tag="ps",
                                           name=f"ps_{qc}_{p}_{hp}_{kb}")
                              for hp in range(2)}
                        for s, lo, hi in segs_of(q0):
                            for hp in range(2):
                                hoff = hp * 64
                                nc.tensor.matmul(
                                    ps[hp][:, lo:hi],
                                    lhsT=kT_sb[hoff:hoff + 64, hb,
                                               kb * KB:(kb + 1) * KB],
                                    rhs=qT_sb[hoff:hoff + 64, hb,
                                              qc * QC + lo:qc * QC + hi],
                                    start=True, stop=True)
                        exs = {}
                        for hp in range(2):
                            if mask is not None:
                                off, c0, wm = mask
                                nc.vector.tensor_add(
                                    ps[hp][:, c0:c0 + wm],
                                    ps[hp][:, c0:c0 + wm],
                                    masks_sb[:, off:off + wm])
                            ex = ex_pool.tile([128, QC], BF16, tag="ex",
                                              name=f"ex_{qc}_{p}_{hp}_{kb}")
                            nc.scalar.activation(
                                out=ex[:, q0:], in_=ps[hp][:, q0:],
                                func=ExpFn, scale=0.125,
                                bias=kbias_sb[:, kb:kb + 1])
                            exs[hp] = ex
                        if pend is not None:
                            pkb, pq0, pexs = pend
                            for hp in range(2):
                                emit_attnv(pa[hp], pkb, pq0, pexs[hp],
                                           seg_first, seg_last, p * 2 + hp)
                        pend = (kb, q0, exs)
                    pkb, pq0, pexs = pend
                    for hp in range(2):
                        emit_attnv(pa[hp], pkb, pq0, pexs[hp],
                                   seg_first, seg_last, p * 2 + hp)
                    for hp in range(2):
                        nc.vector.tensor_copy(araw[:, hp, :], pa[hp][0:64, :])
                        stmp = ship_pool.tile([65, QC], F32, tag="stmp",
                                              name=f"stmp_{qc}_{p}_{hp}")
                        nc.vector.tensor_copy(stmp[64:65, :],
                                              pa[hp][64:65, :])
                        agf = ag_in[(p, qc)].bitcast(F32)
                        for s in range(2):
                            nc.gpsimd.dma_start(
                                out=bass.AP(tensor=agf,
                                            offset=128 * 512 +
                                            (hp * 2 + s) * 512,
                                            ap=[[512, 1], [1, 512]]),
                                in_=stmp[64:65, s * 512:(s + 1) * 512])

                def emit_ship(qc, p, araw):
                    for hp in range(2):
                        nc.gpsimd.dma_start(
                            out=bass.AP(tensor=ag_in[(p, qc)],
                                        offset=hp * 64 * QC,
                                        ap=[[QC, 64], [1, QC]]),
                            in_=araw[:, hp, :])
                    nc.gpsimd.collective_compute(
                        "AllGather", mybir.AluOpType.bypass,
                        replica_groups=GROUPS,
                        ins=[ag_in[(p, qc)][:]], outs=[ag_out[p][qc]])

                # ---- ramp: chunks 0,1 then qc0 interleaved with chunks 2,3
                emit_load(0)
                nc.sync.dma_start(out=masks_sb[:], in_=masks_in[:])
                nc.sync.dma_start(out=kbias_sb[:], in_=kbias_in[:])
                emit_load(1)
                nc.sync.dma_start(out=wv_sb[:], in_=wvT.ap().rearrange(
                    "(db p) c -> p db c", p=128))
                nc.sync.dma_start(out=bv_sb[:], in_=bv_in[:])
                emit_proj_qk(0, wk_sb, bk_sb, kT_sb, "xtk")
                emit_proj_qk(0, wq_sb, bq_sb, qT_sb, "xtq")
                emit_proj_v(0)
                emit_load(2)
                emit_proj_qk(1, wk_sb, bk_sb, kT_sb, "xtk")
                emit_proj_qk(1, wq_sb, bq_sb, qT_sb, "xtq")
                emit_proj_v(1)
                emit_load(3)
                nc.sync.dma_start(out=bo_sb[:], in_=bo_in[:])
                # wo load on the gpsimd queue (after the hot ramp)
                nc.gpsimd.dma_start(out=wo_sb[:], in_=woT.ap().rearrange(
                    "(db p) c -> p db c", p=128))

                araw00 = ship_pool.tile([64, 2, QC], BF16, tag="araw",
                                        name="araw_0_0")
                emit_proj_qk(2, wk_sb, bk_sb, kT_sb, "xtk")
                emit_substream(0, 0, 0, psA0, araw00)
                emit_proj_qk(2, wq_sb, bq_sb, qT_sb, "xtq")
                emit_substream(0, 0, 1, psA0, araw00)
                emit_ship(0, 0, araw00)
                emit_proj_v(2)
                araw01 = ship_pool.tile([64, 2, QC], BF16, tag="araw",
                                        name="araw_0_1")
                emit_substream(0, 1, 0, psA0, araw01)
                emit_proj_qk(3, wk_sb, bk_sb, kT_sb, "xtk")
                emit_proj_qk(3, wq_sb, bq_sb, qT_sb, "xtq")
                emit_substream(0, 1, 1, psA0, araw01)
                emit_ship(0, 1, araw01)
                emit_proj_v(3)

                psA0.release()
                psP.release()
                psA1 = tc.alloc_tile_pool(name="psA1", bufs=2, space="PSUM")

                araw10 = ship_pool.tile([64, 2, QC], BF16, tag="araw",
                                        name="araw_1_0")
                emit_substream_paired(1, 0, psA1, araw10)
                emit_ship(1, 0, araw10)
                # prefetch p=0 phase-O slices on the idle sync queue
                # (j - j%2) == 2*qc_own, so multiply by HALF the qc stride
                off_fat_s = (j_s - j_s % 2) * (2 * AGW * QC) + \
                    (j_s % 2) * 512
                off_s16_s = (j_s - j_s % 2) * (2 * AGW * 512) + \
                    128 * 512 + (j_s % 2) * 512
                fat0 = oph.tile([128, 4, 512], BF16, tag="fat0")
                nc.sync.dma_start(
                    out=fat0[:],
                    in_=bass.AP(tensor=ag_out[0], offset=off_fat_s,
                                ap=[[QC, 128], [AGW * QC, 4], [1, 512]]))
                # S rows fetched as [128, 32] (partition = (r, hp, col/32))
                # so the reciprocal uses all 128 DVE lanes
                s16_0 = oph.tile([128, 32], F32, tag="s16_0")
                for r in range(4):
                    nc.sync.dma_start(
                        out=s16_0[r * 32:(r + 1) * 32, :],
                        in_=bass.AP(tensor=ag_out_f32[0],
                                    offset=off_s16_s + r * (AGW * 512),
                                    ap=[[1024, 2], [32, 16], [1, 32]]))
                araw11 = ship_pool.tile([64, 2, QC], BF16, tag="araw",
                                        name="araw_1_1")
                emit_substream_paired(1, 1, psA1, araw11)
                emit_ship(1, 1, araw11)

                # ---- phase O prep (p=0 normalization, p=1 fetch)
                r16_0 = oph.tile([128, 32], F32, tag="r16_0")
                nc.vector.reciprocal(r16_0[:], s16_0[:])
                nc.scalar.dma_start(
                    out=bass.AP(tensor=r_dram[0], offset=0,
                                ap=[[512, 8], [32, 16], [1, 32]]),
                    in_=r16_0[:])
                bc0 = oph.tile([128, 4, 512], F32, tag="bc0")
                for hpp in range(2):
                    nc.scalar.dma_start(
                        out=bc0[hpp * 64:(hpp + 1) * 64, :, :],
                        in_=bass.AP(tensor=r_dram[0], offset=hpp * 512,
                                    ap=[[0, 64], [1024, 4], [1, 512]]))
                fatn0 = oph.tile([128, 4, 512], BF16, tag="fatn0")
                nc.vector.tensor_mul(
                    fatn0.rearrange("p r l -> p (r l)"),
                    fat0[:].rearrange("p r l -> p (r l)"),
                    bc0[:].rearrange("p r l -> p (r l)"))

                off_fat_c = (j_c - j_c % 2) * (2 * AGW * QC) + \
                    (j_c % 2) * 512
                off_s16_c = (j_c - j_c % 2) * (2 * AGW * 512) + \
                    128 * 512 + (j_c % 2) * 512
                fat1 = oph.tile([128, 4, 512], BF16, tag="fat1")
                nc.scalar.dma_start(
                    out=fat1[:],
                    in_=bass.AP(tensor=ag_out[1], offset=off_fat_c,
                                ap=[[QC, 128], [AGW * QC, 4], [1, 512]]))
                s16_1 = oph.tile([128, 32], F32, tag="s16_1")
                for r in range(4):
                    nc.scalar.dma_start(
                        out=s16_1[r * 32:(r + 1) * 32, :],
                        in_=bass.AP(tensor=ag_out_f32[1],
                                    offset=off_s16_c + r * (AGW * 512),
                                    ap=[[1024, 2], [32, 16], [1, 32]]))
                r16_1 = oph.tile([128, 32], F32, tag="r16_1")
                nc.vector.reciprocal(r16_1[:], s16_1[:])
                nc.scalar.dma_start(
                    out=bass.AP(tensor=r_dram[1], offset=0,
                                ap=[[512, 8], [32, 16], [1, 32]]),
                    in_=r16_1[:])
                bc1 = oph.tile([128, 4, 512], F32, tag="bc1")
                for hpp in range(2):
                    nc.scalar.dma_start(
                        out=bc1[hpp * 64:(hpp + 1) * 64, :, :],
                        in_=bass.AP(tensor=r_dram[1], offset=hpp * 512,
                                    ap=[[0, 64], [1024, 4], [1, 512]]))
                fatn1 = oph.tile([128, 4, 512], BF16, tag="fatn1")
                nc.vector.tensor_mul(
                    fatn1.rearrange("p r l -> p (r l)"),
                    fat1[:].rearrange("p r l -> p (r l)"),
                    bc1[:].rearrange("p r l -> p (r l)"))

                psA1.release()
                psS.release()

                # ---- phase O matmuls
                with tc.tile_pool(name="ob", bufs=4) as obp, \
                     tc.tile_pool(name="psO", bufs=8,
                                  space="PSUM") as psO:
                    po_t = {}
                    for ls in range(4):
                        for nch in range(2):
                            po = psO.tile([128, 512], F32, tag="po",
                                          name=f"po_{ls}_{nch}")
                            po_t[(ls, nch)] = po
                            for r in range(4):
                                nc.tensor.matmul(
                                    po[:],
                                    lhsT=fatn0[:, r,
                                               ls * 128:(ls + 1) * 128],
                                    rhs=wo_sb[:, r * 2,
                                              nch * 512:(nch + 1) * 512],
                                    start=(r == 0), stop=False)
                    for ls in range(4):
                        for nch in range(2):
                            po = po_t[(ls, nch)]
                            for r in range(4):
                                nc.tensor.matmul(
                                    po[:],
                                    lhsT=fatn1[:, r,
                                               ls * 128:(ls + 1) * 128],
                                    rhs=wo_sb[:, r * 2 + 1,
                                              nch * 512:(nch + 1) * 512],
                                    start=False, stop=False)
                            nc.tensor.matmul(
                                po[:], lhsT=ones_sb[:, 0:128],
                                rhs=bo_sb[:, nch * 512:(nch + 1) * 512],
                                start=False, stop=True)
                            ob = obp.tile([128, 512], F32, tag="ob",
                                          name=f"ob_{ls}_{nch}")
                            if nch == 0:
                                nc.vector.tensor_copy(ob[:], po[:])
                            else:
                                nc.scalar.copy(ob[:], po[:])
                            nc.sync.dma_start(
                                out=out[ls * 128:(ls + 1) * 128,
                                        nch * 512:(nch + 1) * 512],
                                in_=ob[:])

    nc.compile()
    return nc


def _host_fallback(query, key, value, attn_mask, key_padding_mask,
                   Wq, bq, Wk, bk, Wv, bv, Wo, bo):
    """Exact fp32 numpy replica of the reference (degenerate masks only)."""
    q = (query @ Wq.T + bq).reshape(B, L, H, DH).transpose(0, 2, 1, 3)
    k = (key @ Wk.T + bk).reshape(B, L, H, DH).transpose(0, 2, 1, 3)
    v = (value @ Wv.T + bv).reshape(B, L, H, DH).transpose(0, 2, 1, 3)
    scores = np.einsum('bhqd,bhkd->bhqk', q, k) / np.sqrt(np.float32(DH))
    scores = np.where(key_padding_mask[:, None, None, :], -1e30, scores)
    scores = np.where(attn_mask[None, None, :, :], -1e30, scores)
    scores = scores - scores.max(axis=-1, keepdims=True)
    w = np.exp(scores)
    w = w / w.sum(axis=-1, keepdims=True)
    attn = np.einsum('bhqk,bhkd->bhqd', w, v)
    attn = attn.transpose(0, 2, 1, 3).reshape(B, L, D)
    return (attn @ Wo.T + bo).astype(np.float32)


def _chunk_input(x):
    """[L, D] fp32 -> [NLC, 128, NDB*512] bf16 chunk-major layout:
    chunk[lc][p][db*512 + l] = x[lc*512 + l, db*128 + p]."""
    xT = np.ascontiguousarray(x.T)                     # [D, L]
    a = xT.reshape(NDB, 128, NLC, 512)                 # [db, p, lc, l]
    return np.ascontiguousarray(
        a.transpose(2, 1, 0, 3).reshape(NLC, 128, NDB * 512)).astype(NPBF16)


def kernel(query, key, value, attn_mask, key_padding_mask,
           Wq, bq, Wk, bk, Wv, bv, Wo, bo):
    global last_results
    query = np.asarray(query, dtype=np.float32)
    key = np.asarray(key, dtype=np.float32)
    value = np.asarray(value, dtype=np.float32)
    attn_mask = np.asarray(attn_mask, dtype=bool)
    key_padding_mask = np.asarray(key_padding_mask, dtype=bool)
    Wq, bq = np.asarray(Wq, np.float32), np.asarray(bq, np.float32)
    Wk, bk = np.asarray(Wk, np.float32), np.asarray(bk, np.float32)
    Wv, bv = np.asarray(Wv, np.float32), np.asarray(bv, np.float32)
    Wo, bo = np.asarray(Wo, np.float32), np.asarray(bo, np.float32)

    structure, mask_buf, kbias, degenerate = _analyze_masks(attn_mask,
                                                            key_padding_mask)
    if degenerate:
        return _host_fallback(query, key, value, attn_mask, key_padding_mask,
                              Wq, bq, Wk, bk, Wv, bv, Wo, bo)

    mw = mask_buf.shape[1]
    key_sig = _structure_key(structure, mw)
    if key_sig not in _PROG_CACHE:
        _PROG_CACHE[key_sig] = _build_program(structure, mw)
    nc = _PROG_CACHE[key_sig]

    woT_np = np.ascontiguousarray(Wo.T).astype(NPBF16)
    bo_np = bo.reshape(1, D).astype(NPBF16)
    xq_b = [_chunk_input(query[b]) for b in range(B)]
    xk_b = [_chunk_input(key[b]) for b in range(B)]
    xv_b = [_chunk_input(value[b]) for b in range(B)]

    in_maps = []
    for core in range(N_CORES):
        b, j = divmod(core, 4)
        csl = slice(j * CPC, (j + 1) * CPC)
        in_maps.append({
            "xq_c": xq_b[b],
            "xk_c": xk_b[b],
            "xv_c": xv_b[b],
            "wqT": np.ascontiguousarray(Wq[csl, :].T).astype(NPBF16),
            "wkT": np.ascontiguousarray(Wk[csl, :].T).astype(NPBF16),
            "wvT": np.ascontiguousarray(Wv[csl, :].T).astype(NPBF16),
            "woT": woT_np,
            "bq": np.ascontiguousarray(bq[csl].reshape(2, 128).T),
            "bk": np.ascontiguousarray(bk[csl].reshape(2, 128).T),
            "bv": bv[csl].reshape(1, CPC).astype(NPBF16),
            "bo": bo_np,
            "masks": mask_buf,
            "kbias": kbias[b],
        })

    trace = os.environ.get("KERNEL_TRACE", "0") == "1"
    res = run_bass_kernel_spmd(nc, in_maps, list(range(N_CORES)), trace=trace)
    last_results = res

    out = np.empty((B, L, D), dtype=np.float32)
    for core in range(N_CORES):
        b, j = divmod(core, 4)
        out[b, j * LPC:(j + 1) * LPC, :] = res.results[core]["out"]
    return out
